# revision 7
# baseline (speedup 1.0000x reference)
"""Trainium2 Bass kernel for AnalyticalCatastropheDetector.

Strategy (8-core pure data parallel):
 - Host: transpose each batch shard to feature-major, stack two halves on
   the partition axis -> x2 [128, NCOLS].  All device DMAs are contiguous.
 - mm1: block-diag(W1,W1) stationary, x2 moving -> h1 stacked [128, N].
 - gelu on ACT (bias b1 fused), full 128 lanes.
 - mm2: g-tile stationary, block-diag Weff moving -> C row-major, where
   Weff = W2 @ Wh (folded, gelu-linear), columns permuted/duplicated into
   31 "blocks" ordered so head math uses contiguous mega-ops.
 - PSUM->SBUF copy deinterleaves C into block-major layout and adds the
   folded bias beta = b2 @ Wh + bh (one scalar_tensor_tensor per chunk).
 - Head math: DVE STT/TT chains + one mega Exp + 7 Tanh on ACT
   (sigmoid(v) = (1+tanh(v/2))/2, outputs scaled by 2, host divides).
   All ACT funcs in phase 2 live in the exp_and_others table set ->
   exactly one table switch after the gelu phase.
"""

import numpy as np

B = 1_000_000
D = 64
NCORE = 8
BC = B // NCORE          # 125000 rows per core
HALF = BC // 2           # 62500
CHUNK = 1024             # x2 columns per chunk (= 2048 rows)
NCH = (HALF + CHUNK - 1) // CHUNK          # 62 chunks
NCOLS = NCH * CHUNK                        # 63488 (padded)
NB = 31                  # coefficient blocks after permutation/duplication
GPC = 2 * (CHUNK // 128)                   # 16 groups (of 128 rows) per chunk
NG = NCH * GPC                             # 992 groups per core
SUBS = [16, 16, 16, 14]                    # chunks per head sub-batch
assert sum(SUBS) == NCH

# Original coefficient indices (order in the reference's 29-col C):
# fold a=0 b=1 | cusp a=2 b=3 c=4 | swal a=5 b=6 c=7 d=8
# butt a=9 b=10 c=11 d=12 e=13 | hyp a=14 b=15 c=16 | ell a=19 b=20 c=21
# par a=24 b=25 c=26 d=27 e=28   (17,18,22,23 unused)
# Block table: (source_coeff, scale) per block position.
# pos 0-9:  plain-abs group; pos 10-16: square group; pos 17-30: raw group.
BLOCKS = [
    (1, 1.0), (3, 1.0), (4, 1.0), (7, 1.0), (8, 1.0),    # 0-4
    (13, 1.0), (27, 1.0), (28, 1.0), (14, 1.0), (19, 1.0),  # 5-9
    (2, 1.0), (5, 1.0), (9, 1.0), (10, 1.0), (14, 1.0),  # 10-14 (square grp)
    (19, 1.0), (26, 0.5),                                # 15-16
    (0, 1.0), (6, 1.0), (11, 1.0), (12, 1.0),            # 17-20
    (15, 1.0), (16, 1.0), (20, 1.0), (21, 1.0),          # 21-24
    (24, 1.0), (25, 1.0), (9, 1.0), (10, 1.0),           # 25-28
    (14, 1.0), (19, 1.0),                                # 29-30
]
assert len(BLOCKS) == NB


def _build_bass(wsoft):
    import sys
    sys.path.insert(0, "/opt/trn_rl_repo")
    from concourse import bass, bacc, mybir
    from concourse.tile import TileContext

    F32 = mybir.dt.float32
    AF = mybir.ActivationFunctionType
    OP = mybir.AluOpType

    nc = bacc.Bacc()
    x2 = nc.declare_dram_parameter("x2", [128, NCOLS], F32, isOutput=False)
    w1bd = nc.declare_dram_parameter("w1bd", [128, 128], F32, isOutput=False)
    b1bd = nc.declare_dram_parameter("b1bd", [128, 1], F32, isOutput=False)
    weffbd = nc.declare_dram_parameter("weffbd", [128, 2 * NB], F32, isOutput=False)
    betat = nc.declare_dram_parameter("betat", [128, GPC // 2 * 2 * NB], F32, isOutput=False)
    cbias = nc.declare_dram_parameter("cbias", [128, 6], F32, isOutput=False)
    outr = nc.declare_dram_parameter("outr", [128, 7 * NG], F32, isOutput=True)
    outt = nc.declare_dram_parameter("outt", [128, NG], F32, isOutput=True)

    w = [float(v) for v in wsoft]

    with TileContext(nc) as tc:
        with (
            tc.tile_pool(name="const", bufs=1) as cpool,
            tc.tile_pool(name="xin", bufs=3) as xpool,
            tc.tile_pool(name="g", bufs=3) as gpool,
            tc.tile_pool(name="p1", bufs=2, space="PSUM") as p1pool,
            tc.tile_pool(name="pc", bufs=2, space="PSUM") as pcpool,
            tc.tile_pool(name="csb", bufs=2) as csbpool,
            tc.tile_pool(name="scr", bufs=1) as scr,
            tc.tile_pool(name="outp", bufs=2) as opool,
        ):
            w1t = cpool.tile([128, 128], F32, tag="w1")
            nc.sync.dma_start(out=w1t[:], in_=w1bd[:])
            b1t = cpool.tile([128, 1], F32, tag="b1")
            nc.sync.dma_start(out=b1t[:], in_=b1bd[:])
            wet = cpool.tile([128, 2 * NB], F32, tag="weff")
            nc.sync.dma_start(out=wet[:], in_=weffbd[:])
            bet = cpool.tile([128, GPC // 2 * 2 * NB], F32, tag="beta")
            nc.sync.dma_start(out=bet[:], in_=betat[:])
            cbt = cpool.tile([128, 6], F32, tag="cbias")
            nc.sync.dma_start(out=cbt[:], in_=cbias[:])
            CB = {0.5: cbt[:, 0:1], 0.25: cbt[:, 1:2], 0.15: cbt[:, 2:3],
                  0.1: cbt[:, 3:4], -0.25: cbt[:, 4:5], -0.5: cbt[:, 5:6]}

            roff = 0  # running col offset into outr
            toff = 0
            gch = 0   # global chunk index
            for sb_chunks in SUBS:
                T = sb_chunks * GPC
                csb = csbpool.tile([128, NB * T], F32, tag="csb")
                c3 = csb[:].rearrange("p (b t) -> p b t", b=NB)
                for ci in range(sb_chunks):
                    xoff = gch * CHUNK
                    xt = xpool.tile([128, CHUNK], F32, tag="xt")
                    nc.sync.dma_start(out=xt[:], in_=x2[:, xoff:xoff + CHUNK])
                    p1 = p1pool.tile([128, CHUNK], F32, tag="p1")
                    nc.tensor.matmul(p1[:, 0:512], w1t[:], xt[:, 0:512],
                                     start=True, stop=True)
                    nc.tensor.matmul(p1[:, 512:1024], w1t[:], xt[:, 512:1024],
                                     start=True, stop=True)
                    g = gpool.tile([128, CHUNK], F32, tag="g")
                    nc.scalar.activation(g[:], p1[:], AF.Gelu,
                                         bias=b1t[:, 0:1], scale=1.0)
                    pc = pcpool.tile([128, (GPC // 2) * 2 * NB], F32, tag="pc")
                    for m in range(GPC // 2):
                        nc.tensor.matmul(
                            pc[:, m * 2 * NB:(m + 1) * 2 * NB],
                            g[:, m * 128:(m + 1) * 128],
                            wet[:], start=True, stop=True)
                    # deinterleave copy + beta add:
                    # pc col = m*(2NB) + s*NB + b  ->  csb[b, ci*GPC + m*2 + s]
                    src = pc[:].rearrange("p (m s b) -> p b m s", m=GPC // 2, s=2)
                    bsrc = bet[:].rearrange("p (m s b) -> p b m s", m=GPC // 2, s=2)
                    dst = c3[:, :, ci * GPC:(ci + 1) * GPC].rearrange(
                        "p b (m s) -> p b m s", s=2)
                    nc.vector.scalar_tensor_tensor(
                        dst, src, 1.0, bsrc, op0=OP.mult, op1=OP.add)
                    gch += 1

                # ---- head math over csb [128, NB*T] ----
                def blk(i):
                    return csb[:, i * T:(i + 1) * T]

                def nt(tagname, width=1):
                    return scr.tile([128, width * T], F32, tag=tagname,
                                    name=tagname)

                ABS = nt("ABS", 10)
                nc.vector.scalar_tensor_tensor(
                    ABS[:], csb[:, 0:10 * T], -1.0, csb[:, 0:10 * T],
                    op0=OP.mult, op1=OP.max)
                SQ = nt("SQ", 7)
                nc.scalar.activation(SQ[:], csb[:, 10 * T:17 * T], AF.Square)
                A = [ABS[:, i * T:(i + 1) * T] for i in range(10)]
                Q = [SQ[:, i * T:(i + 1) * T] for i in range(7)]
                X = [blk(i) for i in range(NB)]
                U = nt("U", 7)
                Uh = [U[:, h * T:(h + 1) * T] for h in range(7)]
                stt = nc.vector.scalar_tensor_tensor
                tt = nc.vector.tensor_tensor
                ts = nc.vector.tensor_scalar

                def negabs(out, x):
                    stt(out, x, -1.0, x, op0=OP.mult, op1=OP.min)

                # fold: U0 = -|a|
                negabs(Uh[0], X[17])
                # cusp: U1 = -|a^2 - 3|b||
                t1 = nt("t1")
                stt(t1[:], A[1], -3.0, Q[0], op0=OP.mult, op1=OP.add)
                negabs(Uh[1], t1[:])
                # swal: U2 = -|c| - |a^2-4b|
                t2 = nt("t2")
                stt(t2[:], X[18], -4.0, Q[1], op0=OP.mult, op1=OP.add)
                t3 = nt("t3")
                negabs(t3[:], t2[:])
                stt(Uh[2], A[3], -1.0, t3[:], op0=OP.mult, op1=OP.add)
                # butt: U3 = -(|d| + |c-ab/2| + |a^3-4.5ac+3.375b^2|)/3
                ab = nt("ab")
                tt(ab[:], X[27], X[28], op=OP.mult)
                tb2 = nt("tb2")
                stt(tb2[:], ab[:], -0.5, X[19], op0=OP.mult, op1=OP.add)
                a3 = nt("a3")
                tt(a3[:], Q[2], X[27], op=OP.mult)
                ac = nt("ac")
                tt(ac[:], X[27], X[19], op=OP.mult)
                tb3 = nt("tb3")
                stt(tb3[:], ac[:], -4.5, a3[:], op0=OP.mult, op1=OP.add)
                tb4 = nt("tb4")
                stt(tb4[:], Q[3], 3.375, tb3[:], op0=OP.mult, op1=OP.add)
                n1 = nt("n1")
                negabs(n1[:], tb4[:])
                n2 = nt("n2")
                negabs(n2[:], tb2[:])
                dn = nt("dn")
                negabs(dn[:], X[20])
                n12 = nt("n12")
                tt(n12[:], n1[:], n2[:], op=OP.add)
                n123 = nt("n123")
                tt(n123[:], n12[:], dn[:], op=OP.add)
                ts(Uh[3], n123[:], 1.0 / 3.0, None, op0=OP.mult)
                # hyp: U4 = -|a^3 - 27bc|/10
                a3h = nt("a3h")
                tt(a3h[:], Q[4], X[29], op=OP.mult)
                bch = nt("bch")
                tt(bch[:], X[21], X[22], op=OP.mult)
                t1h = nt("t1h")
                stt(t1h[:], bch[:], -27.0, a3h[:], op0=OP.mult, op1=OP.add)
                u4n = nt("u4n")
                negabs(u4n[:], t1h[:])
                ts(Uh[4], u4n[:], 0.1, None, op0=OP.mult)
                # ell: U5 = -|a^3 + 27bc|/10
                a3e = nt("a3e")
                tt(a3e[:], Q[5], X[30], op=OP.mult)
                bce = nt("bce")
                tt(bce[:], X[23], X[24], op=OP.mult)
                t1e = nt("t1e")
                stt(t1e[:], bce[:], 27.0, a3e[:], op0=OP.mult, op1=OP.add)
                u5n = nt("u5n")
                negabs(u5n[:], t1e[:])
                ts(Uh[5], u5n[:], 0.1, None, op0=OP.mult)
                # par: U6 = -|ab - c^2/4|
                abp = nt("abp")
                tt(abp[:], X[25], X[26], op=OP.mult)
                t1p = nt("t1p")
                tt(t1p[:], abp[:], Q[6], op=OP.subtract)
                negabs(Uh[6], t1p[:])
                pde = nt("pde")
                tt(pde[:], A[6], A[7], op=OP.add)

                EX = nt("EX", 7)
                nc.scalar.activation(EX[:], U[:], AF.Exp)
                TH = nt("TH", 7)
                Th = [TH[:, h * T:(h + 1) * T] for h in range(7)]
                act = nc.scalar.activation
                # tanh(v/2) for sigmoid args v
                act(Th[0], A[0], AF.Tanh, scale=-0.5, bias=CB[0.5])    # -|b|+1
                act(Th[1], A[2], AF.Tanh, scale=-0.5, bias=CB[0.25])   # -|c|+0.5
                act(Th[2], A[4], AF.Tanh, scale=-0.5, bias=CB[0.15])   # -|d|+0.3
                act(Th[3], A[5], AF.Tanh, scale=-0.5, bias=CB[0.1])    # -|e|+0.2
                act(Th[4], A[8], AF.Tanh, scale=0.5, bias=CB[-0.25])   # |a|-0.5
                act(Th[5], A[9], AF.Tanh, scale=-0.5, bias=CB[-0.25])  # -|a|-0.5
                act(Th[6], pde[:], AF.Tanh, scale=0.5, bias=CB[-0.5])  # |d|+|e|-1

                # R2_h = (tanh+1) * EX_h  == 2 * risk_h
                R2 = opool.tile([128, 7 * T], F32, tag="r2")
                for h in range(7):
                    stt(R2[:, h * T:(h + 1) * T], Th[h], 1.0,
                        EX[:, h * T:(h + 1) * T], op0=OP.add, op1=OP.mult)
                TOT = opool.tile([128, T], F32, tag="tot")
                ts(TOT[:], R2[:, 0:T], w[0], None, op0=OP.mult)
                for h in range(1, 7):
                    stt(TOT[:], R2[:, h * T:(h + 1) * T], w[h], TOT[:],
                        op0=OP.mult, op1=OP.add)
                nc.sync.dma_start(out=outr[:, roff:roff + 7 * T], in_=R2[:])
                nc.sync.dma_start(out=outt[:, toff:toff + T], in_=TOT[:])
                roff += 7 * T
                toff += T
    nc.compile()
    return nc


def kernel(embedding, W1, b1, W2, b2, Wh, bh, g2_weights):
    import sys
    sys.path.insert(0, "/opt/trn_rl_repo")
    from concourse.bass_utils import run_bass_kernel_spmd

    embedding = np.asarray(embedding, np.float32)
    W1 = np.asarray(W1, np.float32)
    b1 = np.asarray(b1, np.float32)
    W2 = np.asarray(W2, np.float32)
    b2 = np.asarray(b2, np.float32)
    Wh = np.asarray(Wh, np.float32)
    bh = np.asarray(bh, np.float32)
    g2 = np.asarray(g2_weights, np.float64)

    e = np.exp(g2 - g2.max())
    wsoft = (e / e.sum()).astype(np.float64)

    weff = (W2 @ Wh).astype(np.float32)            # [64, 29]
    beta = (b2 @ Wh + bh).astype(np.float32)       # [29]
    # permuted/duplicated/scaled blocks
    wcols = np.stack([weff[:, c] * s for c, s in BLOCKS], axis=1)  # [64, NB]
    bvec = np.array([beta[c] * s for c, s in BLOCKS], np.float32)  # [NB]
    weffbd = np.zeros((128, 2 * NB), np.float32)
    weffbd[0:64, 0:NB] = wcols
    weffbd[64:128, NB:2 * NB] = wcols
    w1bd = np.zeros((128, 128), np.float32)
    w1bd[0:64, 0:64] = W1
    w1bd[64:128, 64:128] = W1
    b1bd = np.concatenate([b1, b1]).reshape(128, 1).astype(np.float32)
    # beta tile matching pc layout: col = m*(2NB) + s*NB + b
    brow = np.zeros(((GPC // 2) * 2 * NB,), np.float32)
    for m in range(GPC // 2):
        for s in range(2):
            brow[m * 2 * NB + s * NB: m * 2 * NB + (s + 1) * NB] = bvec
    betat = np.broadcast_to(brow, (128, brow.size)).copy()

    nc = _build_bass(wsoft)

    in_maps = []
    for i in range(NCORE):
        shard = embedding[i * BC:(i + 1) * BC]          # [125000, 64]
        xt = np.ascontiguousarray(shard.T)              # [64, 125000]
        x2 = np.zeros((128, NCOLS), np.float32)
        x2[0:64, 0:HALF] = xt[:, 0:HALF]
        x2[64:128, 0:HALF] = xt[:, HALF:2 * HALF]
        cb = np.broadcast_to(np.array([0.5, 0.25, 0.15, 0.1, -0.25, -0.5],
                                      np.float32), (128, 6)).copy()
        in_maps.append({"x2": x2, "w1bd": w1bd, "b1bd": b1bd,
                        "weffbd": weffbd, "betat": betat, "cbias": cb})

    import os
    trace = bool(os.environ.get("BASS_KERNEL_TRACE"))
    tmpdir = os.environ.get("BASS_KERNEL_TRACE_DIR") or None
    res = run_bass_kernel_spmd(nc, in_maps, list(range(NCORE)),
                               trace=trace, tmpdir=tmpdir)
    if trace:
        print(f"HW exec time: {res.exec_time_ns} ns", flush=True)
    results = res.results

    total = np.empty((B,), np.float32)
    risk = np.empty((B, 7), np.float32)
    for i in range(NCORE):
        outr = results[i]["outr"]      # [128, 7*NG]
        outt = results[i]["outt"]      # [128, NG]
        # reassemble groups: global group u (within sub-batch segment):
        # segment b covers chunks, inside: u = ci*GPC + m*2 + s
        rv = np.empty((2, NCOLS, 7), np.float32)   # [half, col, head]
        tv = np.empty((2, NCOLS), np.float32)
        roff = 0
        toff = 0
        gch = 0
        for sb_chunks in SUBS:
            T = sb_chunks * GPC
            seg = outr[:, roff:roff + 7 * T].reshape(128, 7, sb_chunks,
                                                     GPC // 2, 2)
            segt = outt[:, toff:toff + T].reshape(128, sb_chunks, GPC // 2, 2)
            # row within half = (gch+ci)*CHUNK + m*128 + q  ; half = s
            seg = seg.transpose(4, 2, 3, 0, 1)    # [s, ci, m, q, h]
            segt = segt.transpose(3, 1, 2, 0)     # [s, ci, m, q]
            ncols_seg = sb_chunks * CHUNK
            c0 = gch * CHUNK
            rv[:, c0:c0 + ncols_seg] = seg.reshape(2, ncols_seg, 7)
            tv[:, c0:c0 + ncols_seg] = segt.reshape(2, ncols_seg)
            roff += 7 * T
            toff += T
            gch += sb_chunks
        r0 = i * BC
        risk[r0:r0 + HALF] = rv[0, :HALF] * 0.5
        risk[r0 + HALF:r0 + BC] = rv[1, :HALF] * 0.5
        total[r0:r0 + HALF] = tv[0, :HALF] * 0.5
        total[r0 + HALF:r0 + BC] = tv[1, :HALF] * 0.5
    return total, risk


# revision 9
# speedup vs baseline: 1.6005x; 1.6005x over previous
"""Trainium2 Bass kernel for AnalyticalCatastropheDetector.

Strategy (8-core pure data parallel):
 - Host: transpose each batch shard to feature-major, stack two halves on
   the partition axis -> x2 [128, NCOLS].  All device DMAs are contiguous.
 - mm1: block-diag(W1,W1) stationary, x2 moving -> h1 stacked [128, N].
 - gelu on ACT (bias b1 fused), full 128 lanes.
 - mm2: g-tile stationary, block-diag Weff moving -> C row-major, where
   Weff = W2 @ Wh (folded, gelu-linear), columns permuted/duplicated into
   31 "blocks" ordered so head math uses contiguous mega-ops.
 - PSUM->SBUF copy deinterleaves C into block-major layout and adds the
   folded bias beta = b2 @ Wh + bh (one scalar_tensor_tensor per chunk).
 - Head math: DVE STT/TT chains + one mega Exp + 7 Tanh on ACT
   (sigmoid(v) = (1+tanh(v/2))/2, outputs scaled by 2, host divides).
   All ACT funcs in phase 2 live in the exp_and_others table set ->
   exactly one table switch after the gelu phase.
"""

import numpy as np
import os

MM_BF16 = os.environ.get("K_MM_BF16", "1") == "1"
HEAD_BF16 = os.environ.get("K_HEAD_BF16", "1") == "1"

B = 1_000_000
D = 64
NCORE = 8
BC = B // NCORE          # 125000 rows per core
HALF = BC // 2           # 62500
CHUNK = 1024             # x2 columns per chunk (= 2048 rows)
NCH = (HALF + CHUNK - 1) // CHUNK          # 62 chunks
NCOLS = NCH * CHUNK                        # 63488 (padded)
NB = 31                  # coefficient blocks after permutation/duplication
GPC = 2 * (CHUNK // 128)                   # 16 groups (of 128 rows) per chunk
NG = NCH * GPC                             # 992 groups per core
SUBS = [16, 16, 16, 14]                    # chunks per head sub-batch
assert sum(SUBS) == NCH

# Original coefficient indices (order in the reference's 29-col C):
# fold a=0 b=1 | cusp a=2 b=3 c=4 | swal a=5 b=6 c=7 d=8
# butt a=9 b=10 c=11 d=12 e=13 | hyp a=14 b=15 c=16 | ell a=19 b=20 c=21
# par a=24 b=25 c=26 d=27 e=28   (17,18,22,23 unused)
# Block table: (source_coeff, scale) per block position.
# pos 0-9:  plain-abs group; pos 10-16: square group; pos 17-30: raw group.
BLOCKS = [
    (1, 1.0), (3, 1.0), (4, 1.0), (7, 1.0), (8, 1.0),    # 0-4
    (13, 1.0), (27, 1.0), (28, 1.0), (14, 1.0), (19, 1.0),  # 5-9
    (2, 1.0), (5, 1.0), (9, 1.0), (10, 1.0), (14, 1.0),  # 10-14 (square grp)
    (19, 1.0), (26, 0.5),                                # 15-16
    (0, 1.0), (6, 1.0), (11, 1.0), (12, 1.0),            # 17-20
    (15, 1.0), (16, 1.0), (20, 1.0), (21, 1.0),          # 21-24
    (24, 1.0), (25, 1.0), (9, 1.0), (10, 1.0),           # 25-28
    (14, 1.0), (19, 1.0),                                # 29-30
]
assert len(BLOCKS) == NB


def _build_bass(wsoft):
    import sys
    sys.path.insert(0, "/opt/trn_rl_repo")
    from concourse import bass, bacc, mybir
    from concourse.tile import TileContext

    F32 = mybir.dt.float32
    BF16 = mybir.dt.bfloat16
    MDT = BF16 if MM_BF16 else F32
    HDT = BF16 if HEAD_BF16 else F32
    AF = mybir.ActivationFunctionType
    OP = mybir.AluOpType

    nc = bacc.Bacc()
    x2 = nc.declare_dram_parameter("x2", [128, NCOLS], MDT, isOutput=False)
    w1bd = nc.declare_dram_parameter("w1bd", [128, 128], MDT, isOutput=False)
    b1bd = nc.declare_dram_parameter("b1bd", [128, 1], F32, isOutput=False)
    weffbd = nc.declare_dram_parameter("weffbd", [128, 2 * NB], MDT, isOutput=False)
    betat = nc.declare_dram_parameter("betat", [128, GPC // 2 * 2 * NB], F32, isOutput=False)
    cbias = nc.declare_dram_parameter("cbias", [128, 6], F32, isOutput=False)
    outr = nc.declare_dram_parameter("outr", [128, 7 * NG], HDT, isOutput=True)
    outt = nc.declare_dram_parameter("outt", [128, NG], HDT, isOutput=True)

    w = [float(v) for v in wsoft]

    with TileContext(nc) as tc:
        with (
            tc.tile_pool(name="const", bufs=1) as cpool,
            tc.tile_pool(name="xin", bufs=3) as xpool,
            tc.tile_pool(name="g", bufs=3) as gpool,
            tc.tile_pool(name="p1", bufs=2, space="PSUM") as p1pool,
            tc.tile_pool(name="pc", bufs=2, space="PSUM") as pcpool,
            tc.tile_pool(name="csb", bufs=2) as csbpool,
            tc.tile_pool(name="scr", bufs=1) as scr,
            tc.tile_pool(name="outp", bufs=2) as opool,
        ):
            w1t = cpool.tile([128, 128], MDT, tag="w1")
            nc.sync.dma_start(out=w1t[:], in_=w1bd[:])
            b1t = cpool.tile([128, 1], F32, tag="b1")
            nc.sync.dma_start(out=b1t[:], in_=b1bd[:])
            wet = cpool.tile([128, 2 * NB], MDT, tag="weff")
            nc.sync.dma_start(out=wet[:], in_=weffbd[:])
            bet = cpool.tile([128, GPC // 2 * 2 * NB], F32, tag="beta")
            nc.sync.dma_start(out=bet[:], in_=betat[:])
            cbt = cpool.tile([128, 6], F32, tag="cbias")
            nc.sync.dma_start(out=cbt[:], in_=cbias[:])
            CB = {0.5: cbt[:, 0:1], 0.25: cbt[:, 1:2], 0.15: cbt[:, 2:3],
                  0.1: cbt[:, 3:4], -0.25: cbt[:, 4:5], -0.5: cbt[:, 5:6]}

            roff = 0  # running col offset into outr
            toff = 0
            gch = 0   # global chunk index
            for sb_chunks in SUBS:
                T = sb_chunks * GPC
                csb = csbpool.tile([128, NB * T], HDT, tag="csb")
                c3 = csb[:].rearrange("p (b t) -> p b t", b=NB)
                for ci in range(sb_chunks):
                    xoff = gch * CHUNK
                    xt = xpool.tile([128, CHUNK], MDT, tag="xt")
                    nc.sync.dma_start(out=xt[:], in_=x2[:, xoff:xoff + CHUNK])
                    p1 = p1pool.tile([128, CHUNK], F32, tag="p1")
                    nc.tensor.matmul(p1[:, 0:512], w1t[:], xt[:, 0:512],
                                     start=True, stop=True)
                    nc.tensor.matmul(p1[:, 512:1024], w1t[:],
                                     xt[:, 512:1024],
                                     start=True, stop=True)
                    g = gpool.tile([128, CHUNK], MDT, tag="g")
                    nc.scalar.activation(g[:], p1[:], AF.Gelu,
                                         bias=b1t[:, 0:1], scale=1.0)
                    pc = pcpool.tile([128, (GPC // 2) * 2 * NB], F32, tag="pc")
                    for m in range(GPC // 2):
                        nc.tensor.matmul(
                            pc[:, m * 2 * NB:(m + 1) * 2 * NB],
                            g[:, m * 128:(m + 1) * 128],
                            wet[:], start=True, stop=True)
                    # deinterleave copy + beta add:
                    # pc col = m*(2NB) + s*NB + b  ->  csb[b, ci*GPC + m*2 + s]
                    src = pc[:].rearrange("p (m s b) -> p b m s", m=GPC // 2, s=2)
                    bsrc = bet[:].rearrange("p (m s b) -> p b m s", m=GPC // 2, s=2)
                    dst = c3[:, :, ci * GPC:(ci + 1) * GPC].rearrange(
                        "p b (m s) -> p b m s", s=2)
                    nc.vector.scalar_tensor_tensor(
                        dst, src, 1.0, bsrc, op0=OP.mult, op1=OP.add)
                    gch += 1

                # ---- head math over csb [128, NB*T] ----
                def blk(i):
                    return csb[:, i * T:(i + 1) * T]

                def nt(tagname, width=1):
                    return scr.tile([128, width * T], HDT, tag=tagname,
                                    name=tagname)

                ABS = nt("ABS", 10)
                nc.vector.scalar_tensor_tensor(
                    ABS[:], csb[:, 0:10 * T], -1.0, csb[:, 0:10 * T],
                    op0=OP.mult, op1=OP.max)
                SQ = nt("SQ", 7)
                nc.scalar.activation(SQ[:], csb[:, 10 * T:17 * T], AF.Square)
                A = [ABS[:, i * T:(i + 1) * T] for i in range(10)]
                Q = [SQ[:, i * T:(i + 1) * T] for i in range(7)]
                X = [blk(i) for i in range(NB)]
                U = nt("U", 7)
                Uh = [U[:, h * T:(h + 1) * T] for h in range(7)]
                stt = nc.vector.scalar_tensor_tensor
                tt = nc.vector.tensor_tensor
                ts = nc.vector.tensor_scalar

                def negabs(out, x):
                    stt(out, x, -1.0, x, op0=OP.mult, op1=OP.min)

                # fold: U0 = -|a|
                negabs(Uh[0], X[17])
                # cusp: U1 = -|a^2 - 3|b||
                t1 = nt("t1")
                stt(t1[:], A[1], -3.0, Q[0], op0=OP.mult, op1=OP.add)
                negabs(Uh[1], t1[:])
                # swal: U2 = -|c| - |a^2-4b|
                t2 = nt("t2")
                stt(t2[:], X[18], -4.0, Q[1], op0=OP.mult, op1=OP.add)
                t3 = nt("t3")
                negabs(t3[:], t2[:])
                stt(Uh[2], A[3], -1.0, t3[:], op0=OP.mult, op1=OP.add)
                # butt: U3 = -(|d| + |c-ab/2| + |a^3-4.5ac+3.375b^2|)/3
                ab = nt("ab")
                tt(ab[:], X[27], X[28], op=OP.mult)
                tb2 = nt("tb2")
                stt(tb2[:], ab[:], -0.5, X[19], op0=OP.mult, op1=OP.add)
                a3 = nt("a3")
                tt(a3[:], Q[2], X[27], op=OP.mult)
                ac = nt("ac")
                tt(ac[:], X[27], X[19], op=OP.mult)
                tb3 = nt("tb3")
                stt(tb3[:], ac[:], -4.5, a3[:], op0=OP.mult, op1=OP.add)
                tb4 = nt("tb4")
                stt(tb4[:], Q[3], 3.375, tb3[:], op0=OP.mult, op1=OP.add)
                n1 = nt("n1")
                negabs(n1[:], tb4[:])
                n2 = nt("n2")
                negabs(n2[:], tb2[:])
                dn = nt("dn")
                negabs(dn[:], X[20])
                n12 = nt("n12")
                tt(n12[:], n1[:], n2[:], op=OP.add)
                n123 = nt("n123")
                tt(n123[:], n12[:], dn[:], op=OP.add)
                ts(Uh[3], n123[:], 1.0 / 3.0, None, op0=OP.mult)
                # hyp: U4 = -|a^3 - 27bc|/10
                a3h = nt("a3h")
                tt(a3h[:], Q[4], X[29], op=OP.mult)
                bch = nt("bch")
                tt(bch[:], X[21], X[22], op=OP.mult)
                t1h = nt("t1h")
                stt(t1h[:], bch[:], -27.0, a3h[:], op0=OP.mult, op1=OP.add)
                u4n = nt("u4n")
                negabs(u4n[:], t1h[:])
                ts(Uh[4], u4n[:], 0.1, None, op0=OP.mult)
                # ell: U5 = -|a^3 + 27bc|/10
                a3e = nt("a3e")
                tt(a3e[:], Q[5], X[30], op=OP.mult)
                bce = nt("bce")
                tt(bce[:], X[23], X[24], op=OP.mult)
                t1e = nt("t1e")
                stt(t1e[:], bce[:], 27.0, a3e[:], op0=OP.mult, op1=OP.add)
                u5n = nt("u5n")
                negabs(u5n[:], t1e[:])
                ts(Uh[5], u5n[:], 0.1, None, op0=OP.mult)
                # par: U6 = -|ab - c^2/4|
                abp = nt("abp")
                tt(abp[:], X[25], X[26], op=OP.mult)
                t1p = nt("t1p")
                tt(t1p[:], abp[:], Q[6], op=OP.subtract)
                negabs(Uh[6], t1p[:])
                pde = nt("pde")
                tt(pde[:], A[6], A[7], op=OP.add)

                EX = nt("EX", 7)
                nc.scalar.activation(EX[:], U[:], AF.Exp)
                TH = nt("TH", 7)
                Th = [TH[:, h * T:(h + 1) * T] for h in range(7)]
                act = nc.scalar.activation
                # tanh(v/2) for sigmoid args v
                act(Th[0], A[0], AF.Tanh, scale=-0.5, bias=CB[0.5])    # -|b|+1
                act(Th[1], A[2], AF.Tanh, scale=-0.5, bias=CB[0.25])   # -|c|+0.5
                act(Th[2], A[4], AF.Tanh, scale=-0.5, bias=CB[0.15])   # -|d|+0.3
                act(Th[3], A[5], AF.Tanh, scale=-0.5, bias=CB[0.1])    # -|e|+0.2
                act(Th[4], A[8], AF.Tanh, scale=0.5, bias=CB[-0.25])   # |a|-0.5
                act(Th[5], A[9], AF.Tanh, scale=-0.5, bias=CB[-0.25])  # -|a|-0.5
                act(Th[6], pde[:], AF.Tanh, scale=0.5, bias=CB[-0.5])  # |d|+|e|-1

                # R2_h = (tanh+1) * EX_h  == 2 * risk_h
                R2 = opool.tile([128, 7 * T], HDT, tag="r2")
                for h in range(7):
                    stt(R2[:, h * T:(h + 1) * T], Th[h], 1.0,
                        EX[:, h * T:(h + 1) * T], op0=OP.add, op1=OP.mult)
                TOT = opool.tile([128, T], HDT, tag="tot")
                ts(TOT[:], R2[:, 0:T], w[0], None, op0=OP.mult)
                for h in range(1, 7):
                    stt(TOT[:], R2[:, h * T:(h + 1) * T], w[h], TOT[:],
                        op0=OP.mult, op1=OP.add)
                nc.sync.dma_start(out=outr[:, roff:roff + 7 * T], in_=R2[:])
                nc.sync.dma_start(out=outt[:, toff:toff + T], in_=TOT[:])
                roff += 7 * T
                toff += T
    nc.compile()
    return nc


def kernel(embedding, W1, b1, W2, b2, Wh, bh, g2_weights):
    import sys
    sys.path.insert(0, "/opt/trn_rl_repo")
    from concourse.bass_utils import run_bass_kernel_spmd

    embedding = np.asarray(embedding, np.float32)
    W1 = np.asarray(W1, np.float32)
    b1 = np.asarray(b1, np.float32)
    W2 = np.asarray(W2, np.float32)
    b2 = np.asarray(b2, np.float32)
    Wh = np.asarray(Wh, np.float32)
    bh = np.asarray(bh, np.float32)
    g2 = np.asarray(g2_weights, np.float64)

    e = np.exp(g2 - g2.max())
    wsoft = (e / e.sum()).astype(np.float64)

    weff = (W2 @ Wh).astype(np.float32)            # [64, 29]
    beta = (b2 @ Wh + bh).astype(np.float32)       # [29]
    # permuted/duplicated/scaled blocks
    wcols = np.stack([weff[:, c] * s for c, s in BLOCKS], axis=1)  # [64, NB]
    bvec = np.array([beta[c] * s for c, s in BLOCKS], np.float32)  # [NB]
    weffbd = np.zeros((128, 2 * NB), np.float32)
    weffbd[0:64, 0:NB] = wcols
    weffbd[64:128, NB:2 * NB] = wcols
    w1bd = np.zeros((128, 128), np.float32)
    w1bd[0:64, 0:64] = W1
    w1bd[64:128, 64:128] = W1
    b1bd = np.concatenate([b1, b1]).reshape(128, 1).astype(np.float32)
    # beta tile matching pc layout: col = m*(2NB) + s*NB + b
    brow = np.zeros(((GPC // 2) * 2 * NB,), np.float32)
    for m in range(GPC // 2):
        for s in range(2):
            brow[m * 2 * NB + s * NB: m * 2 * NB + (s + 1) * NB] = bvec
    betat = np.broadcast_to(brow, (128, brow.size)).copy()

    nc = _build_bass(wsoft)

    if MM_BF16:
        import ml_dtypes
        mdt = ml_dtypes.bfloat16
    else:
        mdt = np.float32

    in_maps = []
    for i in range(NCORE):
        shard = embedding[i * BC:(i + 1) * BC]          # [125000, 64]
        xt = np.ascontiguousarray(shard.T)              # [64, 125000]
        x2 = np.zeros((128, NCOLS), mdt)
        x2[0:64, 0:HALF] = xt[:, 0:HALF].astype(mdt)
        x2[64:128, 0:HALF] = xt[:, HALF:2 * HALF].astype(mdt)
        cb = np.broadcast_to(np.array([0.5, 0.25, 0.15, 0.1, -0.25, -0.5],
                                      np.float32), (128, 6)).copy()
        in_maps.append({"x2": x2, "w1bd": w1bd.astype(mdt),
                        "b1bd": b1bd, "weffbd": weffbd.astype(mdt),
                        "betat": betat, "cbias": cb})

    import os
    trace = bool(os.environ.get("BASS_KERNEL_TRACE"))
    tmpdir = os.environ.get("BASS_KERNEL_TRACE_DIR") or None
    res = run_bass_kernel_spmd(nc, in_maps, list(range(NCORE)),
                               trace=trace, tmpdir=tmpdir)
    if trace:
        print(f"HW exec time: {res.exec_time_ns} ns", flush=True)
    results = res.results

    total = np.empty((B,), np.float32)
    risk = np.empty((B, 7), np.float32)
    for i in range(NCORE):
        outr = np.asarray(results[i]["outr"], np.float32)   # [128, 7*NG]
        outt = np.asarray(results[i]["outt"], np.float32)    # [128, NG]
        # reassemble groups: global group u (within sub-batch segment):
        # segment b covers chunks, inside: u = ci*GPC + m*2 + s
        rv = np.empty((2, NCOLS, 7), np.float32)   # [half, col, head]
        tv = np.empty((2, NCOLS), np.float32)
        roff = 0
        toff = 0
        gch = 0
        for sb_chunks in SUBS:
            T = sb_chunks * GPC
            seg = outr[:, roff:roff + 7 * T].reshape(128, 7, sb_chunks,
                                                     GPC // 2, 2)
            segt = outt[:, toff:toff + T].reshape(128, sb_chunks, GPC // 2, 2)
            # row within half = (gch+ci)*CHUNK + m*128 + q  ; half = s
            seg = seg.transpose(4, 2, 3, 0, 1)    # [s, ci, m, q, h]
            segt = segt.transpose(3, 1, 2, 0)     # [s, ci, m, q]
            ncols_seg = sb_chunks * CHUNK
            c0 = gch * CHUNK
            rv[:, c0:c0 + ncols_seg] = seg.reshape(2, ncols_seg, 7)
            tv[:, c0:c0 + ncols_seg] = segt.reshape(2, ncols_seg)
            roff += 7 * T
            toff += T
            gch += sb_chunks
        r0 = i * BC
        risk[r0:r0 + HALF] = rv[0, :HALF] * 0.5
        risk[r0 + HALF:r0 + BC] = rv[1, :HALF] * 0.5
        total[r0:r0 + HALF] = tv[0, :HALF] * 0.5
        total[r0 + HALF:r0 + BC] = tv[1, :HALF] * 0.5
    return total, risk
